# revision 18
# baseline (speedup 1.0000x reference)
"""Trainium2 Bass kernel: sparse (masked) attention with L2 row-normalization.

Per batch b (reference semantics, fp32):
    q = x @ Wq.T ; k = x @ Wk.T ; v = x @ Wv.T          # x: [N, D]
    rel[n, m] = (q[n] . k[m]) * adjacency[m, n]          # multiplicative mask
    out[n]    = sum_m rel[n, m] / ||rel[n, :]||_2 * v[m]

Sharding: data-parallel over batch B=8 -> one batch per NeuronCore, no
collectives. adjacency/weights replicated.

Per-core strategy (all matmul operands bf16, fp32 PSUM accumulate):
  - q k^T == x (Wq^T Wk) x^T, so the host precomputes G = Wq^T @ Wk and the
    kernel runs ONE projection xg^T = G^T-contraction instead of separate
    q/k projections; the score matmul's k-side operand is x^T itself
  - host prepacks x^T / G / Wv^T into partition-major fully-contiguous
    layouts; all input DMAs share one FIFO queue in priority order
  - scores are computed transposed: S^T[m, n] = sum_e xT[e,m] xgT[e,n], so
    the mask is adjacency in its NATIVE layout and the AV matmul needs no
    transposes (lhsT = S^T tile, rhs = v tile)
  - row sum-of-squares (a partition-dim reduction) via ones-vector matmuls
    accumulated in PSUM across m-tiles; the 4 chunk-norm matmuls of an
    m-tile are emitted as one batch, one m-tile late, so the single
    ones-LDWEIGHTS and the DVE/ACT chain never stall the PE pipeline
  - 1/||row|| applied as a per-partition scalar on the AV output tiles
"""

from contextlib import ExitStack

import numpy as np
import ml_dtypes

B, N, D = 8, 2048, 512
P = 128  # SBUF partitions
CHUNK = 512  # fp32 free-dim elems per PSUM bank

_cached = {}


def _build(n=N, d=D):
    import concourse.bacc as bacc
    import concourse.mybir as mybir
    import concourse.tile as tile

    f32 = mybir.dt.float32
    bf16 = mybir.dt.bfloat16

    nt = n // P  # key/query 128-tiles
    dt = d // P  # feature 128-tiles
    ch = min(CHUNK, n)  # free-dim chunk size
    nch = n // ch  # chunks over n
    tpc = ch // P  # 128-tiles per chunk

    nc = bacc.Bacc("TRN2", target_bir_lowering=False, debug=False, num_devices=B)

    # host-prepacked: xTp[p, c, t, j] = x.T[t*P+p, c*ch+j]
    xT_h = nc.dram_tensor("xTp", [P, nch, dt, ch], bf16, kind="ExternalInput")
    # host-prepacked: w2[p, 0, t, e] = G[t*P+p, e] (G = Wq.T @ Wk),
    #                 w2[p, 1, t, e] = Wv.T[t*P+p, e]
    w2_h = nc.dram_tensor("w2", [P, 2, dt, d], bf16, kind="ExternalInput")
    adj_h = nc.dram_tensor("adj", [n, n], bf16, kind="ExternalInput")
    out_h = nc.dram_tensor("out", [n, d], f32, kind="ExternalOutput")

    with tile.TileContext(nc) as tc, ExitStack() as ctx:
        sb = ctx.enter_context(tc.tile_pool(name="sb", bufs=1))
        adj_pool = ctx.enter_context(tc.tile_pool(name="adjp", bufs=3))
        sq_pool = ctx.enter_context(tc.tile_pool(name="sqp", bufs=16))
        outp = ctx.enter_context(tc.tile_pool(name="outp", bufs=2))
        psum = ctx.enter_context(tc.tile_pool(name="psum", bufs=7, space="PSUM"))
        pnrm_pool = ctx.enter_context(tc.tile_pool(name="pnrm", bufs=1, space="PSUM"))

        # ---- input loads: priority order, head-critical ones in parallel -
        # first xg group needs all of G + xT chunk 0 -> 4 parallel streams
        h = dt // 2
        g_sb = sb.tile([P, dt, d], bf16, name="g_sb", tag="g_sb")
        nc.sync.dma_start(g_sb[:, :h], w2_h[:, 0, :h])
        nc.scalar.dma_start(g_sb[:, h:], w2_h[:, 0, h:])

        xT_sb = sb.tile([P, nch, dt, ch], bf16, name="xT_sb", tag="xT_sb")
        nc.gpsimd.dma_start(xT_sb[:, 0, :h], xT_h[:, 0, :h])
        nc.sync.dma_start(xT_sb[:, 0, h:], xT_h[:, 0, h:])

        wv_sb = sb.tile([P, dt, d], bf16, name="wv_sb", tag="wv_sb")
        nc.sync.dma_start(wv_sb[:], w2_h[:, 1])
        for c in range(1, nch):
            nc.sync.dma_start(xT_sb[:, c], xT_h[:, c])

        ones = sb.tile([P, 1], bf16, name="ones", tag="ones")
        nc.vector.memset(ones[:], 1.0)

        # PE warm-up during the initial DMA wait: keeps the HAM activity
        # window busy so real matmuls start at full clock
        warm_rhs = sb.tile([P, ch], bf16, name="warm_rhs", tag="warm_rhs")
        nc.vector.memset(warm_rhs[:], 0.0)
        warm_ps = psum.tile([P, ch], f32, name="mm", tag="mm")
        for _ in range(24):
            nc.tensor.matmul(warm_ps[0:1, :], ones[:], warm_rhs[:])

        def xT_slice(e, m):
            # [128, 128] x^T block: feature-stripe e, key-tile m columns
            return xT_sb[:, m // tpc, e, (m % tpc) * P : (m % tpc + 1) * P]

        # ---- projections, chunk-outer so each xT chunk DMA unlocks work ---
        # xgT[e, n] = sum_d G[d, e] xT[d, n]; v[m, d] = sum_e x[m, e] Wv.T[e, d]
        xgT_sb = [
            sb.tile([P, n], bf16, name=f"xgT{e}", tag=f"xgT{e}") for e in range(dt)
        ]
        v_sb = [sb.tile([P, d], bf16, name=f"v{m}", tag=f"v{m}") for m in range(nt)]
        for c in range(nch):
            for e in range(dt):
                pt = psum.tile([P, ch], f32, name="mm", tag="mm")
                for dd in range(dt):
                    nc.tensor.matmul(
                        pt[:],
                        g_sb[:, dd, e * P : (e + 1) * P],
                        xT_sb[:, c, dd, :],
                        start=(dd == 0),
                        stop=(dd == dt - 1),
                    )
                nc.vector.tensor_copy(xgT_sb[e][:, c * ch : (c + 1) * ch], pt[:])
            for m in range(c * tpc, (c + 1) * tpc):
                pt = psum.tile([P, d], f32, name="mm", tag="mm")
                for e in range(dt):
                    nc.tensor.matmul(
                        pt[:],
                        xT_slice(e, m),
                        wv_sb[:, e, :],
                        start=(e == 0),
                        stop=(e == dt - 1),
                    )
                nc.vector.tensor_copy(v_sb[m][:], pt[:])

        # ---- scores + mask + sum-of-squares -------------------------------
        st_sb = [sb.tile([P, n], bf16, name=f"st{m}", tag=f"st{m}") for m in range(nt)]
        # all nch norm accumulators share ONE PSUM bank at partition 32*c
        pnrm = pnrm_pool.tile([P, ch], f32, name="pnrm", tag="pnrm")

        # norm matmuls for m-tile m are emitted as one batch (single
        # ones-LDWEIGHTS) after m+1's scores, so the PE pipeline never breaks
        sq_tiles = {}

        def emit_norm_batch(mm_idx):
            for c in range(nch):
                nc.tensor.matmul(
                    pnrm[32 * c : 32 * c + 1, :],
                    ones[:],
                    sq_tiles.pop((mm_idx, c))[:],
                    start=(mm_idx == 0),
                    stop=(mm_idx == nt - 1),
                    tile_position=(0, 32 * c),
                )

        for m in range(nt):
            adj_t = adj_pool.tile([P, n], bf16, name="adj_t", tag="adj_t")
            nc.sync.dma_start(adj_t[:], adj_h[m * P : (m + 1) * P, :])
            for c in range(nch):
                ps = psum.tile([P, ch], f32, name="mm", tag="mm")
                for e in range(dt):
                    nc.tensor.matmul(
                        ps[:],
                        xT_slice(e, m),
                        xgT_sb[e][:, c * ch : (c + 1) * ch],
                        start=(e == 0),
                        stop=(e == dt - 1),
                    )
                stm = st_sb[m][:, c * ch : (c + 1) * ch]
                nc.vector.tensor_mul(stm, ps[:], adj_t[:, c * ch : (c + 1) * ch])
                sq = sq_pool.tile([P, ch], bf16, name="sq", tag="sq")
                nc.scalar.square(sq[:], stm)
                sq_tiles[m, c] = sq
            if m >= 2 and m % 2 == 0:
                emit_norm_batch(m - 2)
                emit_norm_batch(m - 1)
        emit_norm_batch(nt - 2)
        emit_norm_batch(nt - 1)

        # ---- 1/||row||: sqrt rows (ACT) -> scatter -> per-chunk recip ----
        # per-chunk pieces so AV tile t only waits on its own quarter
        rcp_c = []
        for c in range(nch):
            nrm_row = sb.tile([1, ch], f32, name=f"nrm_row{c}", tag=f"nrm_row{c}")
            nc.scalar.sqrt(nrm_row[:], pnrm[32 * c : 32 * c + 1, :])
            nrm_tc = sb.tile([P, tpc], f32, name=f"nrm_tc{c}", tag=f"nrm_tc{c}")
            # scatter [1, ch] -> [P, tpc] so scale is a per-partition scalar
            for tt in range(tpc):
                eng = nc.gpsimd if tt % 2 == 0 else nc.sync
                eng.dma_start(
                    nrm_tc[:, tt : tt + 1], nrm_row[:, tt * P : (tt + 1) * P]
                )
            rt = sb.tile([P, tpc], f32, name=f"rcp_c{c}", tag=f"rcp_c{c}")
            nc.vector.reciprocal(rt[:], nrm_tc[:])
            rcp_c.append(rt)

        # ---- AV + normalization scale ------------------------------------
        for t in range(nt):
            pav = psum.tile([P, d], f32, name="mm", tag="mm")
            for m in range(nt):
                nc.tensor.matmul(
                    pav[:],
                    st_sb[m][:, t * P : (t + 1) * P],
                    v_sb[m][:],
                    start=(m == 0),
                    stop=(m == nt - 1),
                )
            ot = outp.tile([P, d], f32, name="ot", tag="ot")
            nc.vector.tensor_scalar_mul(
                ot[:], pav[:], rcp_c[t // tpc][:, t % tpc : t % tpc + 1]
            )
            nc.scalar.dma_start(out_h[t * P : (t + 1) * P, :], ot[:])

    nc.compile()
    return nc


def _prep_in_maps(inputs, n=N, d=D):
    bf = ml_dtypes.bfloat16
    dt = d // P
    ch = min(CHUNK, n)
    nch = n // ch

    x = np.asarray(inputs["neuron_states"])
    adj = np.ascontiguousarray(np.asarray(inputs["adjacency"]).astype(bf))
    G = (np.asarray(inputs["Wq"]).T.astype(np.float64) @ np.asarray(inputs["Wk"]).astype(np.float64)).astype(np.float32)
    w_all = np.stack([G, np.asarray(inputs["Wv"]).T]).astype(bf)
    # w2[p, i, t, e] = {G, Wv.T}[t*P+p, e]
    w2 = np.ascontiguousarray(w_all.reshape(2, dt, P, d).transpose(2, 0, 1, 3))
    in_maps = []
    for b in range(x.shape[0]):
        xT = x[b].T.astype(bf)  # [d, n]
        # xTp[p, c, t, j] = xT[t*P+p, c*ch+j]
        xTp = np.ascontiguousarray(xT.reshape(dt, P, nch, ch).transpose(1, 2, 0, 3))
        in_maps.append({"xTp": xTp, "adj": adj, "w2": w2})
    return in_maps


def _run(inputs, trace=False, **kw):
    from concourse.bass_utils import run_bass_kernel_spmd

    if "nc" not in _cached:
        _cached["nc"] = _build()
    in_maps = _prep_in_maps(inputs)
    res = run_bass_kernel_spmd(
        _cached["nc"], in_maps, core_ids=list(range(len(in_maps))), trace=trace, **kw
    )
    out = np.stack([r["out"] for r in res.results], axis=0)
    return out, res


def kernel(**inputs):
    return _run(inputs)[0]
